# revision 1
# baseline (speedup 1.0000x reference)
"""Trainium2 Bass kernel for nn_CayleyConv (gnn_message_passing).

Self-contained: kernel(**inputs) -> np.ndarray [50000, 128] fp32.

Algorithm notes (derived from the reference):
  - Off-diagonal part S of A = hL - iI is REAL (-h * w_norm, row != col).
  - B y = hL y + i y, diag(hL) = d_r, inv_diag = (d_r + i) / (d_r^2 + 1).
  - Only Re(cum) is used => out = x@W0.T + 2*sum_r (Yr_r@Wre[r].T - Yi_r@Wim[r].T).
  - 33 real-weighted SpMVs of the fixed sparse S applied to (yr|yi) planes.

Distribution (8 NeuronCores):
  - Nodes permuted into 8 cores x 49 blocks x 128 slots (LPT-balanced by
    in-degree, block edge-count capped at 4096).
  - Per dest block: edges split into two halves by source-table window
    (int16 gather index limit), each padded to 2048 = 16 chunks of 128.
  - SpMV per block: dma_gather 2x2048 rows from the fp16 Y table, then 32
    one-hot matmuls (host-precomputed M blocks, fp16) accumulate S@[yr|yi]
    into PSUM. Elementwise Jacobi update on DVE, new yk shard AllGathered
    into every core's Y table each iteration.
"""
import heapq
import os
import numpy as np


# ---------------------------------------------------------------- config ----
class Cfg:
    def __init__(self, n=50000, e=1600000, c=128, r=3, njac=10,
                 ncores=8, blocks=49, half_cap=2048):
        self.N, self.E, self.C, self.R, self.NJAC = n, e, c, r, njac
        self.NCORES, self.BLOCKS, self.HALF_CAP = ncores, blocks, half_cap
        self.BLK = 128
        self.SPC = blocks * self.BLK                # slots per core
        self.SLOTS = ncores * self.SPC
        self.CPH = half_cap // 128                  # chunks per half
        self.CHUNKS = 2 * self.CPH
        self.BLOCK_CAP = 2 * half_cap
        self.IDX_MAX = 32767
        self.HALF_B_BASE = max(0, self.SLOTS - 32768)
        assert self.SLOTS - self.HALF_B_BASE <= 32768
        assert self.BLK * blocks * ncores >= n


FULL = Cfg()


# --------------------------------------------------------- preprocessing ----
def preprocess(cfg, x, edge_index, edge_weight, h):
    N, BLK, BLOCKS, NCORES = cfg.N, cfg.BLK, cfg.BLOCKS, cfg.NCORES
    row = np.asarray(edge_index[0], dtype=np.int64)
    col = np.asarray(edge_index[1], dtype=np.int64)
    w = np.asarray(edge_weight, dtype=np.float64)
    x = np.asarray(x, dtype=np.float32)
    h0 = float(np.asarray(h).reshape(-1)[0])

    deg = np.bincount(row, weights=w, minlength=N)
    dis = np.where(deg > 0, deg ** -0.5, 0.0)
    wn = dis[row] * w * dis[col]

    sl = row == col
    d_r = h0 * (1.0 - np.bincount(row[sl], weights=wn[sl], minlength=N))
    pv = d_r / (d_r ** 2 + 1.0)
    qv = 1.0 / (d_r ** 2 + 1.0)

    er, ec, ew = row[~sl], col[~sl], (-h0 * wn[~sl])

    # LPT: nodes -> bins (core, block), balance in-degree, cap edges per bin
    indeg = np.bincount(er, minlength=N)
    order = np.argsort(-indeg, kind="stable")
    nbins = NCORES * BLOCKS
    heap = [(0, b) for b in range(nbins)]
    heapq.heapify(heap)
    bin_count = np.zeros(nbins, dtype=np.int64)
    g = np.empty(N, dtype=np.int64)
    for v in order:
        dv = int(indeg[v])
        popped = []
        while True:
            load, b = heapq.heappop(heap)
            if bin_count[b] < BLK and load + dv <= cfg.BLOCK_CAP:
                break
            popped.append((load, b))
        g[v] = b * BLK + bin_count[b]
        bin_count[b] += 1
        if bin_count[b] < BLK:
            heapq.heappush(heap, (load + dv, b))
        for it in popped:
            heapq.heappush(heap, it)

    node_of_slot = np.full(cfg.SLOTS, -1, dtype=np.int64)
    node_of_slot[g] = np.arange(N)

    es, src = g[er], g[ec]
    e_bin, e_dl = es // BLK, es % BLK
    must_b = src > cfg.IDX_MAX
    must_a = src < cfg.HALF_B_BASE
    flexible = ~must_a & ~must_b

    idx_all = np.zeros((NCORES, BLOCKS, 2, cfg.HALF_CAP), dtype=np.int16)
    m_all = np.zeros((NCORES, BLOCKS, BLK, cfg.CHUNKS, BLK), dtype=np.float16)

    order_e = np.argsort(e_bin, kind="stable")
    bstart = np.searchsorted(e_bin[order_e], np.arange(nbins + 1))
    for b in range(nbins):
        core, blk = divmod(b, BLOCKS)
        sel = order_e[bstart[b]:bstart[b + 1]]
        mb, fl = must_b[sel], flexible[sel]
        na_must, nb_must, nfl = int((~mb & ~fl).sum()), int(mb.sum()), int(fl.sum())
        lo = max(0, nfl + nb_must - cfg.HALF_CAP)
        hi = min(nfl, cfg.HALF_CAP - na_must)
        assert lo <= hi, f"bin {b} half-split infeasible"
        n_to_a = (lo + hi) // 2
        fl_idx = sel[fl]
        for hf, lst in ((0, np.concatenate([sel[~mb & ~fl], fl_idx[:n_to_a]])),
                        (1, np.concatenate([sel[mb], fl_idx[n_to_a:]]))):
            k = len(lst)
            srcs = src[lst] - (cfg.HALF_B_BASE if hf else 0)
            idx_all[core, blk, hf, :k] = srcs.astype(np.int16)
            j = np.arange(k)
            ch = hf * cfg.CPH + j // 128
            m_all[core, blk, j % 128, ch, e_dl[lst]] = ew[lst].astype(np.float16)

    # per-slot diag vectors [core][lane, block]
    dpq = np.zeros((NCORES, BLK, 3 * BLOCKS), dtype=np.float32)
    s_core, s_rem = g // cfg.SPC, g % cfg.SPC
    s_blk, s_lane = s_rem // BLK, s_rem % BLK
    dpq[s_core, s_lane, s_blk] = d_r
    dpq[s_core, s_lane, BLOCKS + s_blk] = pv
    dpq[s_core, s_lane, 2 * BLOCKS + s_blk] = qv

    # initial table
    y0 = np.zeros((cfg.SLOTS, 2 * cfg.C), dtype=np.float32)
    y0[g, :cfg.C] = x
    Y0 = y0.astype(np.float16)

    # idx sbuf wrap layout [128, BLOCKS*2*(HALF_CAP//16)]
    F = cfg.HALF_CAP // 16
    wrap = idx_all.reshape(NCORES, BLOCKS, 2, F, 16).transpose(0, 4, 1, 2, 3)
    wrap = wrap.reshape(NCORES, 16, BLOCKS * 2 * F)
    idx_sb = np.tile(wrap, (1, 8, 1))  # replicate to 128 partitions

    m_dram = m_all.reshape(NCORES, BLOCKS, BLK, cfg.CHUNKS * BLK)
    return dict(g=g, node_of_slot=node_of_slot, idx_sb=idx_sb, m_dram=m_dram,
                dpq=dpq, Y0=Y0, h0=h0)


def make_wts(cfg, W0, Wre, Wim):
    """[128, (2+2R)*128] fp32: W0T, WreT[r], -WimT[r], identity (host layout)."""
    C = cfg.C
    mats = [np.asarray(W0, np.float32).T]
    for r_ in range(cfg.R):
        mats.append(np.asarray(Wre[r_], np.float32).T)
        mats.append(-np.asarray(Wim[r_], np.float32).T)
    mats.append(np.eye(C, dtype=np.float32))
    return np.concatenate(mats, axis=1)  # [128, (2R+2)*128]


# ------------------------------------------------------------ bass kernel ---
def build_nc(cfg):
    import concourse.bacc as bacc
    import concourse.mybir as mybir
    import concourse.tile as tile
    from concourse.library_config import mlp

    fp16, fp32, i16 = mybir.dt.float16, mybir.dt.float32, mybir.dt.int16
    Alu = mybir.AluOpType
    C, C2, BLK, NB = cfg.C, 2 * cfg.C, cfg.BLK, cfg.BLOCKS
    HC, CPH, CH = cfg.HALF_CAP, cfg.CPH, cfg.CHUNKS
    F = HC // 16
    NW = 2 + 2 * cfg.R

    nc = bacc.Bacc("TRN2", target_bir_lowering=False, debug=False,
                   num_devices=cfg.NCORES, num_swdge_queues=4)

    Y0 = nc.dram_tensor("y0_in", [cfg.SLOTS, C2], fp16, kind="ExternalInput")
    YSH = nc.dram_tensor("yshard_in", [cfg.SPC, C2], fp16, kind="ExternalInput")
    MB = nc.dram_tensor("m_in", [NB, BLK, CH * BLK], fp16, kind="ExternalInput")
    IDX = nc.dram_tensor("idx_in", [128, NB * 2 * F], i16, kind="ExternalInput")
    DPQ = nc.dram_tensor("dpq_in", [128, 3 * NB], fp32, kind="ExternalInput")
    WTS = nc.dram_tensor("wts_in", [128, NW * C], fp32, kind="ExternalInput")
    OUT = nc.dram_tensor("out", [cfg.SPC, C], fp32, kind="ExternalOutput")

    with tile.TileContext(nc) as tc:
        nc.gpsimd.load_library(mlp)
        import contextlib
        with contextlib.ExitStack() as ctx:
            dram = ctx.enter_context(tc.tile_pool(name="dram", bufs=1, space="DRAM"))
            persist = ctx.enter_context(tc.tile_pool(name="persist", bufs=1))
            gp = ctx.enter_context(tc.tile_pool(name="gp", bufs=3))
            mp = ctx.enter_context(tc.tile_pool(name="mp", bufs=3))
            sp = ctx.enter_context(tc.tile_pool(name="sp", bufs=3))
            pp = ctx.enter_context(
                tc.tile_pool(name="pp", bufs=2, space="PSUM"))
            pt = ctx.enter_context(
                tc.tile_pool(name="pt", bufs=2, space="PSUM"))

            ytab = dram.tile([cfg.SLOTS, C2], fp16)
            agin = dram.tile([cfg.SPC, C2], fp16)

            idx_sb = persist.tile([128, NB * 2 * F], i16)
            dpq_sb = persist.tile([128, 3 * NB], fp32)
            wts_sb = persist.tile([128, NW * C], fp32)
            b_sb = persist.tile([128, NB * C2], fp32)
            acc_sb = persist.tile([128, NB * C], fp32)
            zero_sb = persist.tile([128, C], fp32)
            nc.vector.memset(zero_sb[:], 0.0)

            nc.sync.dma_start(idx_sb[:], IDX[:])
            nc.sync.dma_start(dpq_sb[:], DPQ[:])
            nc.sync.dma_start(wts_sb[:], WTS[:])
            nc.sync.dma_start(ytab[:], Y0[:])
            nc.sync.dma_start(agin[:], YSH[:])

            ident = wts_sb[:, (NW - 1) * C:NW * C]
            tabA = ytab[0:min(32768, cfg.SLOTS), :]
            tabB = ytab[cfg.HALF_B_BASE:cfg.SLOTS, :]
            qn = [0]

            def spmv_psum(cb):
                """Gathers + one-hot matmuls for block cb -> psum tile."""
                m_tile = mp.tile([128, CH * BLK], fp16, name="m_tile", tag="m")
                nc.sync.dma_start(m_tile[:], MB[cb, :, :])
                g_tile = gp.tile([128, CH, C2], fp16, name="g_tile", tag="g")
                for hf in range(2):
                    off = (cb * 2 + hf) * F
                    tab = tabB if hf else tabA
                    nc.gpsimd.dma_gather(
                        g_tile[:, hf * CPH:(hf + 1) * CPH, :], tab,
                        idx_sb[:, off:off + F], HC, HC, C2,
                        single_packet=False, queue_num=qn[0] & 3)
                    qn[0] += 1
                psum = pp.tile([128, C2], fp32, name="psum_sy", tag="psy")
                for c_ in range(CH):
                    nc.tensor.matmul(
                        psum[:], m_tile[:, c_ * BLK:(c_ + 1) * BLK],
                        g_tile[:, c_, :], start=(c_ == 0), stop=(c_ == CH - 1))
                return psum

            def jacobi_update(cb, psum, t_r, t_i):
                """yk = inv_diag * (t_r + i t_i) -> fp16 -> agin rows."""
                p_col = dpq_sb[:, NB + cb:NB + cb + 1]
                q_col = dpq_sb[:, 2 * NB + cb:2 * NB + cb + 1]
                a1 = sp.tile([128, C], fp32, name="a1", tag="a1")
                a2 = sp.tile([128, C], fp32, name="a2", tag="a2")
                yk = sp.tile([128, C2], fp16, name="yk", tag="yk")
                nc.vector.scalar_tensor_tensor(
                    a1[:], t_i, q_col, zero_sb[:], Alu.mult, Alu.add)
                nc.vector.scalar_tensor_tensor(
                    yk[:, 0:C], t_r, p_col, a1[:], Alu.mult, Alu.subtract)
                nc.vector.scalar_tensor_tensor(
                    a2[:], t_r, q_col, zero_sb[:], Alu.mult, Alu.add)
                nc.vector.scalar_tensor_tensor(
                    yk[:, C:C2], t_i, p_col, a2[:], Alu.mult, Alu.add)
                nc.sync.dma_start(agin[cb * BLK:(cb + 1) * BLK, :], yk[:])

            def b_pass():
                for cb in range(NB):
                    psum = spmv_psum(cb)
                    d_col = dpq_sb[:, cb:cb + 1]
                    y_t = sp.tile([128, C2], fp16, name="y_t", tag="yt")
                    nc.sync.dma_start(y_t[:], agin[cb * BLK:(cb + 1) * BLK, :])
                    w1 = sp.tile([128, C], fp32, name="w1", tag="w1")
                    w2 = sp.tile([128, C], fp32, name="w2", tag="w2")
                    b_r = b_sb[:, cb * C2:cb * C2 + C]
                    b_i = b_sb[:, cb * C2 + C:(cb + 1) * C2]
                    nc.vector.scalar_tensor_tensor(
                        w1[:], y_t[:, 0:C], d_col, y_t[:, C:C2],
                        Alu.mult, Alu.subtract)
                    nc.vector.tensor_add(b_r, w1[:], psum[:, 0:C])
                    nc.vector.scalar_tensor_tensor(
                        w2[:], y_t[:, C:C2], d_col, y_t[:, 0:C],
                        Alu.mult, Alu.add)
                    nc.vector.tensor_add(b_i, w2[:], psum[:, C:C2])
                    jacobi_update(cb, psum, b_r, b_i)

            def j_pass():
                for cb in range(NB):
                    psum = spmv_psum(cb)
                    t = sp.tile([128, C2], fp32, name="t", tag="t")
                    nc.vector.tensor_sub(
                        t[:], b_sb[:, cb * C2:(cb + 1) * C2], psum[:])
                    jacobi_update(cb, psum, t[:, 0:C], t[:, C:C2])

            def acc_pass(r_):
                # r_ == -1: acc = y_r @ W0T ; else acc += 2*(yrT.T@WreT - yiT.T@WimT)
                for cb in range(NB):
                    y_t = sp.tile([128, C2], fp16, name="y_acc", tag="yacc")
                    nc.sync.dma_start(y_t[:], agin[cb * BLK:(cb + 1) * BLK, :])
                    acc = acc_sb[:, cb * C:(cb + 1) * C]
                    planes = (1,) if r_ < 0 else (0, 1)
                    pso = pp.tile([128, C], fp32, name="psum_o", tag="pso")
                    for k, pl in enumerate(planes if r_ >= 0 else (0,)):
                        y32 = sp.tile([128, C], fp32, name="y32", tag="y32")
                        nc.vector.scalar_tensor_tensor(
                            y32[:], y_t[:, pl * C:(pl + 1) * C], 1.0,
                            zero_sb[:], Alu.mult, Alu.add)
                        pstr = pt.tile([128, C], fp32, name="pstr", tag="pstr")
                        nc.tensor.transpose(pstr[:], y32[:], ident)
                        yT = sp.tile([128, C], fp32, name="yT", tag="yT")
                        nc.vector.tensor_copy(yT[:], pstr[:])
                        wsl = 0 if r_ < 0 else (1 + 2 * r_ + pl)
                        nc.tensor.matmul(
                            pso[:], yT[:], wts_sb[:, wsl * C:(wsl + 1) * C],
                            start=(k == 0), stop=(k == len(planes) - 1 or r_ < 0))
                    if r_ < 0:
                        nc.vector.tensor_copy(acc, pso[:])
                    else:
                        nc.vector.scalar_tensor_tensor(
                            acc, pso[:], 2.0, acc, Alu.mult, Alu.add)

            def allgather():
                nc.gpsimd.collective_compute(
                    "AllGather", Alu.bypass,
                    replica_groups=[list(range(cfg.NCORES))],
                    ins=[agin.opt()], outs=[ytab.opt()])

            acc_pass(-1)
            for r_ in range(cfg.R):
                b_pass()
                allgather()
                for j_ in range(cfg.NJAC):
                    j_pass()
                    if not (r_ == cfg.R - 1 and j_ == cfg.NJAC - 1):
                        allgather()
                acc_pass(r_)

            for cb in range(NB):
                nc.sync.dma_start(OUT[cb * BLK:(cb + 1) * BLK, :],
                                  acc_sb[:, cb * C:(cb + 1) * C])

    nc.compile()
    return nc


_NC_CACHE = {}


def _get_nc(cfg):
    key = (cfg.N, cfg.E, cfg.BLOCKS, cfg.HALF_CAP, cfg.R, cfg.NJAC)
    if key not in _NC_CACHE:
        _NC_CACHE[key] = build_nc(cfg)
    return _NC_CACHE[key]


def run_on_device(cfg, pp, wts, trace=False):
    from concourse.bass_utils import run_bass_kernel_spmd
    nc = _get_nc(cfg)
    in_maps = []
    for core in range(cfg.NCORES):
        sh0 = pp["Y0"][core * cfg.SPC:(core + 1) * cfg.SPC]
        in_maps.append(dict(
            y0_in=pp["Y0"], yshard_in=np.ascontiguousarray(sh0),
            m_in=pp["m_dram"][core], idx_in=pp["idx_sb"][core],
            dpq_in=pp["dpq"][core], wts_in=wts))
    res = run_bass_kernel_spmd(nc, in_maps, core_ids=list(range(cfg.NCORES)),
                               trace=trace)
    outs = np.stack([res.results[c]["out"] for c in range(cfg.NCORES)])
    return outs.reshape(cfg.SLOTS, cfg.C), res


def kernel(x, edge_index, edge_weight, h, W0, Wre, Wim):
    cfg = FULL
    pp = preprocess(cfg, x, edge_index, edge_weight, h)
    wts = make_wts(cfg, W0, Wre, Wim)
    flat, _ = run_on_device(cfg, pp, wts,
                            trace=bool(int(os.environ.get("KTRACE", "0"))))
    out = np.zeros((cfg.N, cfg.C), dtype=np.float32)
    nos = pp["node_of_slot"]
    valid = nos >= 0
    out[nos[valid]] = flat[valid]
    return out

